# revision 2
# baseline (speedup 1.0000x reference)
"""Trainium2 Bass kernel for the LIF spiking block — v2.

Pipeline per (batch, channel-group) tile [128ch x 1024t], 16 tiles/core,
batch-sharded over 8 cores (2 batches each):

  DVE   : membrane scan (f32, in-place over current)
          cumsum1 scan  (spikes bf16 -> A bf16; fp32 state => A==small exact)
          cumsum2 scan  (A bf16 -> z bf16; z==1 detection stays exact)
  Act   : spike = Relu(Sign(m - v_th))        (2 passes, bf16)
          out   = Relu(1 - Abs(z - 1))        (2 passes, exact for integer z)
          membrane bf16 downconvert           (1 pass)
  SP    : current f32 in; membrane/z/out bf16 out (18MB/core vs 32MB f32)

The DVE runs one iteration ahead of its own cumsums so the Act spike
compare for tile i overlaps the cumsums of tile i-1.

bf16 error budget: z & membrane ~0.2% (<< 2e-2 gate); `out` exact because
the scan state is fp32 and bf16 rounds integers monotonically, so
bf16(z)==1 iff z==1.
"""

import os
import numpy as np

B_FULL, C, T = 16, 1024, 1024
N_CORES = 8
B_SHARD = B_FULL // N_CORES  # 2
P = 128
NG = C // P  # 8
NITER = B_SHARD * NG  # 16
NBUF = 5

_PROGRAM_CACHE = {}
LAST_RESULTS = None


def _build_program():
    import concourse.bass as bass
    from concourse import mybir

    f32 = mybir.dt.float32
    bf16 = mybir.dt.bfloat16
    op = mybir.AluOpType
    AF = mybir.ActivationFunctionType

    nc = bass.Bass()

    cur_d = nc.declare_dram_parameter("current", [B_SHARD, C, T], f32, isOutput=False)
    beta_d = nc.declare_dram_parameter("beta_t", [P, NG], f32, isOutput=False)
    vinit_d = nc.declare_dram_parameter("vinit_t", [P, B_SHARD, NG], f32, isOutput=False)
    nvth_d = nc.declare_dram_parameter("nvth_t", [P, B_SHARD, NG], f32, isOutput=False)
    out_d = nc.declare_dram_parameter("out", [B_SHARD, C, T], bf16, isOutput=True)
    z_d = nc.declare_dram_parameter("z", [B_SHARD, C, T], bf16, isOutput=True)
    mem_d = nc.declare_dram_parameter("membrane", [B_SHARD, C, T], bf16, isOutput=True)

    from contextlib import ExitStack

    with ExitStack() as st:
        block = st.enter_context(nc.Block())
        s_prm = st.enter_context(nc.semaphore("s_prm"))
        s_cur = [st.enter_context(nc.semaphore(f"s_cur{j}")) for j in range(NBUF)]
        s_stm = [st.enter_context(nc.semaphore(f"s_stm{j}")) for j in range(NBUF)]
        s_stz = [st.enter_context(nc.semaphore(f"s_stz{j}")) for j in range(NBUF)]
        s_sto = [st.enter_context(nc.semaphore(f"s_sto{j}")) for j in range(NBUF)]
        s_set = st.enter_context(nc.semaphore("s_set"))  # const setup (DVE)
        s_mem = st.enter_context(nc.semaphore("s_mem"))  # membrane scan done
        s_spk = st.enter_context(nc.semaphore("s_spk"))  # spike relu done
        s_c1 = st.enter_context(nc.semaphore("s_c1"))    # cumsum1 done
        s_z = st.enter_context(nc.semaphore("s_z"))      # cumsum2 done
        s_oo = st.enter_context(nc.semaphore("s_oo"))    # out relu done
        s_cvt = st.enter_context(nc.semaphore("s_cvt"))  # membrane bf16 copy done

        cur_sb = st.enter_context(nc.sbuf_tensor("cur_sb", [P, NBUF, T], f32))
        spk_sb = st.enter_context(nc.sbuf_tensor("spk_sb", [P, NBUF, T], bf16))
        a_sb = st.enter_context(nc.sbuf_tensor("a_sb", [P, NBUF, T], bf16))
        z_sb = st.enter_context(nc.sbuf_tensor("z_sb", [P, NBUF, T], bf16))
        oo_sb = st.enter_context(nc.sbuf_tensor("oo_sb", [P, NBUF, T], bf16))
        mb_sb = st.enter_context(nc.sbuf_tensor("mb_sb", [P, NBUF, T], bf16))
        sgn_sb = st.enter_context(nc.sbuf_tensor("sgn_sb", [P, T], bf16))
        t2_sb = st.enter_context(nc.sbuf_tensor("t2_sb", [P, T], bf16))
        beta_sb = st.enter_context(nc.sbuf_tensor("beta_sb", [P, NG], f32))
        vinit_sb = st.enter_context(nc.sbuf_tensor("vinit_sb", [P, B_SHARD, NG], f32))
        nvth_sb = st.enter_context(nc.sbuf_tensor("nvth_sb", [P, B_SHARD, NG], f32))
        one_sb = st.enter_context(nc.sbuf_tensor("one_sb", [P, 1], f32))
        neg1_sb = st.enter_context(nc.sbuf_tensor("neg1_sb", [P, 1], f32))
        zero_sb = st.enter_context(nc.sbuf_tensor("zero_sb", [P, 1], f32))

        def iter_slices(i):
            b, g = divmod(i, NG)
            c0, c1 = g * P, (g + 1) * P
            return b, g, c0, c1, i % NBUF

        @block.sync
        def _(sp):
            sp.dma_start(out=beta_sb[:], in_=beta_d[:]).then_inc(s_prm, 16)
            sp.dma_start(out=vinit_sb[:], in_=vinit_d[:]).then_inc(s_prm, 16)
            sp.dma_start(out=nvth_sb[:], in_=nvth_d[:]).then_inc(s_prm, 16)
            for i in range(NITER):
                b, g, c0, c1, sl = iter_slices(i)
                if i >= 2:
                    j = i - 2
                    jb, jg, jc0, jc1, jsl = iter_slices(j)
                    sp.wait_ge(s_cvt, j + 1)
                    sp.dma_start(
                        out=mem_d[jb, jc0:jc1, :], in_=mb_sb[:, jsl, :]
                    ).then_inc(s_stm[jsl], 16)
                    sp.wait_ge(s_z, j + 1)
                    sp.dma_start(
                        out=z_d[jb, jc0:jc1, :], in_=z_sb[:, jsl, :]
                    ).then_inc(s_stz[jsl], 16)
                    sp.wait_ge(s_oo, j + 1)
                    sp.dma_start(
                        out=out_d[jb, jc0:jc1, :], in_=oo_sb[:, jsl, :]
                    ).then_inc(s_sto[jsl], 16)
            for j in (NITER - 2, NITER - 1):
                jb, jg, jc0, jc1, jsl = iter_slices(j)
                sp.wait_ge(s_cvt, j + 1)
                sp.dma_start(out=mem_d[jb, jc0:jc1, :], in_=mb_sb[:, jsl, :]).then_inc(
                    s_stm[jsl], 16
                )
                sp.wait_ge(s_z, j + 1)
                sp.dma_start(out=z_d[jb, jc0:jc1, :], in_=z_sb[:, jsl, :]).then_inc(
                    s_stz[jsl], 16
                )
                sp.wait_ge(s_oo, j + 1)
                sp.dma_start(out=out_d[jb, jc0:jc1, :], in_=oo_sb[:, jsl, :]).then_inc(
                    s_sto[jsl], 16
                )

        @block.gpsimd
        def _(pool):
            for i in range(NITER):
                b, g, c0, c1, sl = iter_slices(i)
                if i >= NBUF:
                    # cur slot readers from tile i-NBUF: Act Sign then Copy;
                    # Copy is last in Act's per-tile order.
                    pool.wait_ge(s_cvt, i - NBUF + 1)
                pool.dma_start(out=cur_sb[:, sl, :], in_=cur_d[b, c0:c1, :]).then_inc(
                    s_cur[sl], 16
                )

        @block.vector
        def _(vec):
            vec.memset(one_sb[:], 1.0)
            vec.memset(neg1_sb[:], -1.0)
            vec.memset(zero_sb[:], 0.0)
            vec.wait_ge(s_prm, 48)
            vec.then_inc_dummy = None

            def cumsums(j):
                jb, jg, jc0, jc1, jsl = iter_slices(j)
                vec.wait_ge(s_spk, j + 1)
                vec.tensor_tensor_scan(
                    out=a_sb[:, jsl, :],
                    data0=one_sb[:].broadcast_to([P, T]),
                    data1=spk_sb[:, jsl, :],
                    initial=0.0,
                    op0=op.mult,
                    op1=op.add,
                ).then_inc(s_c1, 1)
                if j >= NBUF:
                    # z slot jsl: previous tile j-NBUF's store + Act Abs read
                    vec.wait_ge(s_stz[jsl], 16 * (j // NBUF))
                    vec.wait_ge(s_oo, j - NBUF + 1)
                vec.tensor_tensor_scan(
                    out=z_sb[:, jsl, :],
                    data0=one_sb[:].broadcast_to([P, T]),
                    data1=a_sb[:, jsl, :],
                    initial=0.0,
                    op0=op.mult,
                    op1=op.add,
                ).then_inc(s_z, 1)

            for i in range(NITER):
                b, g, c0, c1, sl = iter_slices(i)
                k = i // NBUF
                vec.wait_ge(s_cur[sl], 16 * (k + 1))
                vec.tensor_tensor_scan(
                    out=cur_sb[:, sl, :],
                    data0=beta_sb[:, g : g + 1].broadcast_to([P, T]),
                    data1=cur_sb[:, sl, :],
                    initial=vinit_sb[:, b, g : g + 1],
                    op0=op.mult,
                    op1=op.add,
                ).then_inc(s_mem, 1)
                if i >= 1:
                    cumsums(i - 1)
            cumsums(NITER - 1)

        @block.scalar
        def _(act):
            def out_eq(j):
                jb, jg, jc0, jc1, jsl = iter_slices(j)
                act.wait_ge(s_z, j + 1)
                act.activation(
                    out=t2_sb[:], in_=z_sb[:, jsl, :],
                    func=AF.Abs, bias=neg1_sb[:], scale=1.0,
                )
                if j >= NBUF:
                    act.wait_ge(s_sto[jsl], 16 * (j // NBUF))
                act.activation(
                    out=oo_sb[:, jsl, :], in_=t2_sb[:],
                    func=AF.Relu, bias=one_sb[:], scale=-1.0,
                ).then_inc(s_oo, 1)

            act.wait_ge(s_prm, 48)
            for i in range(NITER):
                b, g, c0, c1, sl = iter_slices(i)
                k = i // NBUF
                act.wait_ge(s_mem, i + 1)
                act.activation(
                    out=sgn_sb[:], in_=cur_sb[:, sl, :],
                    func=AF.Sign, bias=nvth_sb[:, b, g : g + 1], scale=1.0,
                )
                if i >= NBUF:
                    act.wait_ge(s_c1, i - NBUF + 1)  # spk slot free
                act.activation(
                    out=spk_sb[:, sl, :], in_=sgn_sb[:],
                    func=AF.Relu, bias=zero_sb[:], scale=1.0,
                ).then_inc(s_spk, 1)
                if i >= NBUF:
                    act.wait_ge(s_stm[sl], 16 * k)  # mb slot free
                act.activation(
                    out=mb_sb[:, sl, :], in_=cur_sb[:, sl, :], func=AF.Copy
                ).then_inc(s_cvt, 1)
                if i >= 1:
                    out_eq(i - 1)
            out_eq(NITER - 1)

    return nc


def get_program():
    if "nc" not in _PROGRAM_CACHE:
        _PROGRAM_CACHE["nc"] = _build_program()
    return _PROGRAM_CACHE["nc"]


def _kernel_numpy(current, beta, v_init, v_th):
    """Fallback exact path (only used if v_th varies along t)."""
    cur = current.astype(np.float64).copy()
    cur[:, :, 0] += (beta[None, :] * v_init).astype(np.float32)
    m = np.empty_like(cur)
    state = np.zeros(cur.shape[:2])
    for t in range(cur.shape[2]):
        state = (beta[None, :] * state).astype(np.float32).astype(np.float64) + cur[:, :, t]
        state = state.astype(np.float32).astype(np.float64)
        m[:, :, t] = state
    spk = (m > v_th).astype(np.float64)
    z = np.cumsum(np.cumsum(spk, axis=-1), axis=-1)
    out = np.where(z == 1.0, 1.0, 0.0)
    return (
        out.astype(np.float32),
        z.astype(np.float32),
        m.astype(np.float32),
    )


def kernel(current, beta, v_init, v_th):
    global LAST_RESULTS
    from concourse.bass_utils import run_bass_kernel_spmd

    current = np.ascontiguousarray(current, dtype=np.float32)
    beta = np.ascontiguousarray(beta, dtype=np.float32)
    v_init = np.ascontiguousarray(v_init, dtype=np.float32)
    v_th = np.ascontiguousarray(v_th, dtype=np.float32)

    if not np.all(v_th == v_th[:, :, :1]):
        return _kernel_numpy(current, beta, v_init, v_th)

    nc = get_program()

    beta_t = np.ascontiguousarray(beta.reshape(NG, P).T)
    in_maps = []
    for k in range(N_CORES):
        lo, hi = k * B_SHARD, (k + 1) * B_SHARD
        vinit_t = np.ascontiguousarray(
            v_init[lo:hi].reshape(B_SHARD, NG, P).transpose(2, 0, 1)
        )
        nvth_t = np.ascontiguousarray(
            -v_th[lo:hi, :, 0].reshape(B_SHARD, NG, P).transpose(2, 0, 1)
        )
        in_maps.append(
            {
                "current": np.ascontiguousarray(current[lo:hi]),
                "beta_t": beta_t,
                "vinit_t": vinit_t,
                "nvth_t": nvth_t,
            }
        )

    trace = bool(int(os.environ.get("KERNEL_TRACE", "0")))
    res = run_bass_kernel_spmd(nc, in_maps, list(range(N_CORES)), trace=trace)
    LAST_RESULTS = res

    out = np.concatenate(
        [np.asarray(r["out"], np.float32) for r in res.results], axis=0
    )
    z = np.concatenate([np.asarray(r["z"], np.float32) for r in res.results], axis=0)
    membrane = np.concatenate(
        [np.asarray(r["membrane"], np.float32) for r in res.results], axis=0
    )
    return out, z, membrane
